# revision 2
# baseline (speedup 1.0000x reference)
"""Block-sparse top-k masked linear for Trainium2, tensor-parallel over 8 cores.

out = (block_masked x) @ W + bias
  x: (128, 1, 4096) fp16, W: (4096, 11008) fp16, bias: (11008,) fp16
  mask: per (32-row x 64-col) block of x, keep blocks whose mean |x| is
  >= the 32nd-largest of the 64 k-block activations in that row block.

Sharding: column-parallel -- each of the 8 cores gets an 11008/8 = 1376
column slice of W and bias; x is replicated; outputs are concatenated.

W is relaid out on the host to [128, 32*1376] (partition-major: row p
holds W[kt*128+p, :] for all 32 k-tiles) so each W DMA moves 11KB of
contiguous bytes per partition instead of 2.75KB -- big descriptors keep
the three DMA queues at the ~358 GB/s HBM-per-core limit instead of the
~280 GB/s the row-major layout reaches.
"""
from contextlib import ExitStack

import numpy as np

import concourse.bass as bass
import concourse.tile as tile
from concourse import bacc, mybir
from concourse.bass_utils import run_bass_kernel_spmd

F16 = mybir.dt.float16
F32 = mybir.dt.float32
AX = mybir.AxisListType
ALU = mybir.AluOpType
ACT = mybir.ActivationFunctionType

M = 128          # rows of x
K = 4096         # contraction
N = 11008        # out features
NCORES = 8
NLOC = N // NCORES           # 1376 columns per core
BLOCK_M, BLOCK_K = 32, 64
NBM, NBK = M // BLOCK_M, K // BLOCK_K   # 4 row blocks, 64 k blocks
KEEP = 32                               # k blocks kept per row block
NKT = K // 128                          # 32 k tiles of 128
N_TILES = [(0, 512), (512, 512), (1024, 352)]   # n-tile offsets/sizes

# W chunk schedule: 16 chunks of 2 k-tiles (704KB each).  Queue q's chunks
# drain in order; with the three queues sharing HBM evenly, global arrival
# order is round-robin G,S,A (gpsimd starts first: its queue only carries
# 73KB of consts ahead of W, vs 512KB of x on sync/scalar).  k-tiles are
# assigned to chunks in that arrival order so the GEMM (program order
# kt=0..31) is never blocked on a late chunk.
CHUNK_KT = 2
NCHUNK = NKT // CHUNK_KT
# chunk index (in kt order: chunk i owns kt 2i, 2i+1) -> queue
_G_CHUNKS = [0, 3, 6, 9, 12, 15]
_S_CHUNKS = [1, 4, 7, 10, 13]
_A_CHUNKS = [2, 5, 8, 11, 14]


def _program(ctx: ExitStack, tc: tile.TileContext, ins, outs):
    nc = tc.nc
    x_d, w_d, b_d, e_d, blob_d = ins
    (o_d,) = outs

    const = ctx.enter_context(tc.tile_pool(name="const", bufs=1))
    mk = ctx.enter_context(tc.tile_pool(name="mk", bufs=1))
    xc = ctx.enter_context(tc.tile_pool(name="xc", bufs=2))
    xtp = ctx.enter_context(tc.tile_pool(name="xtp", bufs=1))
    wpool = ctx.enter_context(tc.tile_pool(name="wpool", bufs=NCHUNK))
    xmpool = ctx.enter_context(tc.tile_pool(name="xmpool", bufs=8))
    opool = ctx.enter_context(tc.tile_pool(name="opool", bufs=1))
    psum = ctx.enter_context(tc.tile_pool(name="psum", bufs=1, space="PSUM"))

    # consts first on the gpsimd ring: blob = [ident | jh | ksel], then E,
    # bias.  ident leads because the PE transposes need it earliest.
    blob = const.tile([128, 288], F16)
    nc.gpsimd.dma_start(blob[:], blob_d)
    ident = blob[:, 0:128]
    jh = blob[0:64, 128:256]
    ksel = blob[0:64, 256:256 + NKT]
    e_sb = const.tile([128, NBM], F32)
    nc.gpsimd.dma_start(e_sb[:], e_d)
    bias_sb = const.tile([1, NLOC], F16)
    nc.gpsimd.dma_start(bias_sb[:], b_d)

    # ---- HAM warm-up: ~4us of junk matmuls so the PE clock-gate opens
    # before the transposes/GEMM start (otherwise everything runs at 1.2 GHz)
    warm_sb = mk.tile([128, 512], F16)
    nc.vector.memset(warm_sb[:], 0.0)
    warm_ps = psum.tile([128, 512], F32, name="warm_ps", tag="warm", bufs=1)
    for i in range(9):
        nc.tensor.matmul(warm_ps[:], lhsT=warm_sb[:, 0:128], rhs=warm_sb[:],
                         start=True, stop=True)

    # ---- x in two halves on the HWDGE queues (4KB lines)
    x_cs = []
    for c in range(2):
        x_c = xc.tile([128, K // 2], F16, name=f"xch{c}", tag="xch")
        (nc.sync if c == 0 else nc.scalar).dma_start(
            x_c[:], x_d[:, c * (K // 2):(c + 1) * (K // 2)])
        x_cs.append(x_c)

    # ---- W chunks: 2 k-tiles per DMA, 11KB per partition line
    wch = [None] * NCHUNK
    for ci in _G_CHUNKS:
        wch[ci] = wpool.tile([128, CHUNK_KT * NLOC], F16, name=f"w{ci}", tag="w")
        nc.gpsimd.dma_start(
            wch[ci][:], w_d[:, ci * CHUNK_KT * NLOC:(ci + 1) * CHUNK_KT * NLOC])
    for ci in _S_CHUNKS:
        wch[ci] = wpool.tile([128, CHUNK_KT * NLOC], F16, name=f"w{ci}", tag="w")
        nc.sync.dma_start(
            wch[ci][:], w_d[:, ci * CHUNK_KT * NLOC:(ci + 1) * CHUNK_KT * NLOC])
    for ci in _A_CHUNKS:
        wch[ci] = wpool.tile([128, CHUNK_KT * NLOC], F16, name=f"w{ci}", tag="w")
        nc.scalar.dma_start(
            wch[ci][:], w_d[:, ci * CHUNK_KT * NLOC:(ci + 1) * CHUNK_KT * NLOC])

    # part_n[m, j] = sum_k |x[m, 64 j + k]|
    part_n = mk.tile([128, NBK], F32)
    for c in range(2):
        nc.vector.tensor_reduce(
            part_n[:, c * (NBK // 2):(c + 1) * (NBK // 2)],
            x_cs[c][:].rearrange("p (j k) -> p j k", k=BLOCK_K),
            axis=AX.X, op=ALU.add, apply_absolute_value=True)

    # PE-transpose each 128-wide k tile; batch 4 per PSUM flush into xt_all
    xt_all = xtp.tile([128, K], F16)
    for tb in range(NKT // 4):
        tp = psum.tile([128, 512], F16, name=f"tp{tb}", tag="tp", bufs=2)
        for t in range(4):
            kt = 4 * tb + t
            src = x_cs[kt // 16]
            nc.tensor.transpose(tp[:, t * 128:(t + 1) * 128],
                                src[:, (kt % 16) * 128:(kt % 16 + 1) * 128],
                                ident)
        nc.vector.tensor_copy(xt_all[:, tb * 512:(tb + 1) * 512], tp[:])

    # ba_ps[b, j] = sum_m E[m, b] * part_n[m, j]  (block sums, b on partitions)
    ba_ps = psum.tile([NBM, NBK], F32, tag="mkps", bufs=2)
    nc.tensor.matmul(ba_ps[:], lhsT=e_sb[:], rhs=part_n[:], start=True, stop=True)

    # mean = sum / 2048 (exact power of two), rounded to f16 like jnp.mean
    ba16 = mk.tile([NBM, NBK], F16)
    nc.vector.tensor_scalar_mul(ba16[:], ba_ps[:], 1.0 / 2048.0)

    # arow[i, b*64+j] = a[b, j] on 64 partitions, via block-diag expand + matmul
    # rhs3[c, b*64+j] = a[c, j] * [c == b]
    rhs3 = mk.tile([NBM, NBM * NBK], F16)
    nc.vector.tensor_tensor(
        rhs3[:].rearrange("c (b j) -> c b j", b=NBM),
        ba16[:].unsqueeze(1).broadcast_to((NBM, NBM, NBK)),
        ident[0:NBM, 0:NBM].unsqueeze(-1).broadcast_to((NBM, NBM, NBK)),
        op=ALU.mult)
    ones4c = mk.tile([NBM, 64], F16)
    nc.vector.memset(ones4c[:], 1.0)
    arow_ps = psum.tile([64, NBM * NBK], F32, tag="mkps", bufs=2)
    nc.tensor.matmul(arow_ps[:], lhsT=ones4c[:], rhs=rhs3[:], start=True, stop=True)
    arow = mk.tile([64, NBM * NBK], F16)
    nc.vector.tensor_copy(arow[:], arow_ps[:])

    # acol[i, b] = a[b, i] via PE transpose
    acol_ps = psum.tile([64, NBM], F16, tag="mkps", bufs=2)
    nc.tensor.transpose(acol_ps[:], ba16[:], ident[0:NBM, 0:NBM])
    acol = mk.tile([64, NBM], F16)
    nc.vector.tensor_copy(acol[:], acol_ps[:])

    # cnt[i, b] = #{j : a[b, j] > a[b, i]};  keep iff cnt < KEEP
    cmp = mk.tile([64, NBM * NBK], F16)
    nc.vector.tensor_tensor(
        cmp[:].rearrange("i (b j) -> i b j", b=NBM),
        arow[:].rearrange("i (b j) -> i b j", b=NBM),
        acol[:].unsqueeze(-1).broadcast_to((64, NBM, NBK)),
        op=ALU.is_gt)
    cnt = mk.tile([64, NBM], F32)
    nc.vector.tensor_reduce(cnt[:], cmp[:].rearrange("i (b j) -> i b j", b=NBM),
                            axis=AX.X, op=ALU.add)
    keep16 = mk.tile([64, NBM], F16)
    nc.vector.tensor_scalar(keep16[:], cnt[:], float(KEEP), None, op0=ALU.is_lt)

    # keep2[p, kt*4+b] = keep16[2kt + p//64, b]  (kt-major so a 4-ktile xm
    # batch reads a contiguous 16-column slice)
    # rhs2[j, kt*4+b] = keep16[j, b] * Ksel[j, kt]
    rhs2 = mk.tile([64, 128], F16)
    nc.vector.tensor_tensor(
        rhs2[:].rearrange("j (kt b) -> j kt b", kt=NKT),
        keep16[:].unsqueeze(1).broadcast_to((64, NKT, NBM)),
        ksel[:].unsqueeze(-1).broadcast_to((64, NKT, NBM)),
        op=ALU.mult)
    ks_ps = psum.tile([128, 128], F32, tag="mkps", bufs=2)
    nc.tensor.matmul(ks_ps[:], lhsT=jh[:], rhs=rhs2[:], start=True, stop=True)
    keep2 = mk.tile([128, 128], F16)
    nc.vector.tensor_copy(keep2[:], ks_ps[:])

    ones = const.tile([1, 128], F16)
    nc.vector.memset(ones[:], 1.0)

    # ---- main GEMM: out[m, n] = sum_kt xm_kt.T @ w_kt + ones.T @ bias ----
    pbanks = [psum.tile([128, 512], F32, name=f"pn{i}", tag=f"pn{i}")
              for i in range(3)]
    # bias as the FIRST accumulation into each bank (start=True) so the
    # banks are complete right when the last k-tile matmul lands
    for nt, (n0, nsz) in enumerate(N_TILES):
        nc.tensor.matmul(pbanks[nt][:, :nsz], lhsT=ones[:],
                         rhs=bias_sb[:, n0:n0 + nsz], start=True, stop=False)

    # masked xT in 4-ktile batches: xm[p, q*32+m] = xt * keep2[p, q]
    xms = []
    for xb in range(NKT // 4):
        xm_b = xmpool.tile([128, 512], F16, name=f"xm{xb}", tag="xm")
        nc.vector.tensor_tensor(
            xm_b[:].rearrange("p (q m) -> p q m", m=BLOCK_M),
            xt_all[:, xb * 512:(xb + 1) * 512].rearrange(
                "p (q m) -> p q m", m=BLOCK_M),
            keep2[:, xb * 16:(xb + 1) * 16].unsqueeze(-1).broadcast_to(
                (128, 16, BLOCK_M)),
            op=ALU.mult)
        xms.append(xm_b)

    for kt in range(NKT):
        lhsT = xms[kt // 4][:, (kt % 4) * 128:(kt % 4 + 1) * 128]
        wbase = (kt % CHUNK_KT) * NLOC
        for nt, (n0, nsz) in enumerate(N_TILES):
            nc.tensor.matmul(pbanks[nt][:, :nsz],
                             lhsT=lhsT,
                             rhs=wch[kt // CHUNK_KT][:, wbase + n0:wbase + n0 + nsz],
                             start=False, stop=(kt == NKT - 1))

    out_sb = opool.tile([128, NLOC], F16)
    out_dma = [nc.sync, nc.scalar, nc.gpsimd]
    pi = 0
    for nt, (n0, nsz) in enumerate(N_TILES):
        for half in range(2):
            h0 = n0 + half * (nsz // 2)
            hsz = nsz // 2 if half == 0 else nsz - nsz // 2
            src = pbanks[nt][:, h0 - n0:h0 - n0 + hsz]
            dst = out_sb[:, h0:h0 + hsz]
            if pi % 2 == 0:
                nc.scalar.activation(dst, src, ACT.Copy)
            else:
                nc.vector.tensor_copy(dst, src)
            out_dma[pi % 3].dma_start(o_d[:, h0:h0 + hsz], dst)
            pi += 1


_CACHE = {}


def _build():
    if "nc" in _CACHE:
        return _CACHE["nc"]
    nc = bacc.Bacc("TRN2", target_bir_lowering=False, debug=False,
                   num_devices=NCORES)
    x_d = nc.dram_tensor("x", (M, K), F16, kind="ExternalInput").ap()
    w_d = nc.dram_tensor("w", (128, NKT * NLOC), F16, kind="ExternalInput").ap()
    b_d = nc.dram_tensor("bias", (1, NLOC), F16, kind="ExternalInput").ap()
    e_d = nc.dram_tensor("E", (M, NBM), F32, kind="ExternalInput").ap()
    blob_d = nc.dram_tensor("blob", (128, 288), F16, kind="ExternalInput").ap()
    o_d = nc.dram_tensor("out", (M, NLOC), F16, kind="ExternalOutput").ap()
    with tile.TileContext(nc) as tc:
        with ExitStack() as ctx:
            _program(ctx, tc, [x_d, w_d, b_d, e_d, blob_d], [o_d])
    nc.compile()
    _CACHE["nc"] = nc
    return nc


def _make_in_maps(x2, weight, bias):
    e_np = np.zeros((M, NBM), np.float32)
    for b in range(NBM):
        e_np[b * BLOCK_M:(b + 1) * BLOCK_M, b] = 1.0
    j_idx = np.arange(64)
    blob_np = np.zeros((128, 288), np.float16)
    blob_np[:, 0:128] = np.eye(128, dtype=np.float16)
    blob_np[0:64, 128:256] = (
        j_idx[:, None] % 2 == (np.arange(128)[None, :] // 64)).astype(np.float16)
    blob_np[0:64, 256:256 + NKT] = (
        j_idx[:, None] // 2 == np.arange(NKT)[None, :]).astype(np.float16)

    w16 = np.asarray(weight).astype(np.float16, copy=False)
    b16 = np.asarray(bias).astype(np.float16, copy=False)
    in_maps = []
    for c in range(NCORES):
        sl = slice(c * NLOC, (c + 1) * NLOC)
        # [K, NLOC] -> [128, NKT*NLOC]: row p holds W[kt*128+p, :] for all kt
        wp = np.ascontiguousarray(
            w16[:, sl].reshape(NKT, 128, NLOC).transpose(1, 0, 2).reshape(
                128, NKT * NLOC))
        in_maps.append({
            "x": x2,
            "w": wp,
            "bias": np.ascontiguousarray(b16[sl].reshape(1, NLOC)),
            "E": e_np,
            "blob": blob_np,
        })
    return in_maps


def kernel(x: np.ndarray, weight: np.ndarray, bias: np.ndarray) -> np.ndarray:
    x = np.asarray(x)
    weight = np.asarray(weight)
    bias = np.asarray(bias)
    bsz, seq, hidden = x.shape
    assert (bsz, seq, hidden) == (M, 1, K) and weight.shape == (K, N)

    x2 = np.ascontiguousarray(x.reshape(M, K).astype(np.float16, copy=False))
    in_maps = _make_in_maps(x2, weight, bias)
    nc = _build()
    res = run_bass_kernel_spmd(nc, in_maps, core_ids=list(range(NCORES)))
    out = np.concatenate([r["out"] for r in res.results], axis=1)
    return out.reshape(M, 1, N).astype(x.dtype, copy=False)


if __name__ == "__main__":
    rng = np.random.default_rng(0)
    x = rng.standard_normal((M, 1, K)).astype(np.float16)
    w = (rng.standard_normal((K, N)) * 0.01).astype(np.float16)
    b = np.zeros((N,), np.float16)
    out = kernel(x, w, b)
    print(out.shape, out.dtype)


# revision 6
# speedup vs baseline: 1.0093x; 1.0093x over previous
"""Block-sparse top-k masked linear for Trainium2, tensor-parallel over 8 cores.

out = (block_masked x) @ W + bias
  x: (128, 1, 4096) fp16, W: (4096, 11008) fp16, bias: (11008,) fp16
  mask: per (32-row x 64-col) block of x, keep blocks whose mean |x| is
  >= the 32nd-largest of the 64 k-block activations in that row block.

Sharding: column-parallel -- each of the 8 cores gets an 11008/8 = 1376
column slice of W and bias; x is replicated; outputs are concatenated.

W is relaid out on the host to [128, 32*1376] (partition-major: row p
holds W[kt*128+p, :] for all 32 k-tiles) so each W DMA moves 11KB of
contiguous bytes per partition instead of 2.75KB -- big descriptors keep
the three DMA queues at the ~358 GB/s HBM-per-core limit instead of the
~280 GB/s the row-major layout reaches.
"""
from contextlib import ExitStack

import numpy as np

import concourse.bass as bass
import concourse.tile as tile
from concourse import bacc, mybir
from concourse.bass_utils import run_bass_kernel_spmd

F16 = mybir.dt.float16
F32 = mybir.dt.float32
AX = mybir.AxisListType
ALU = mybir.AluOpType
ACT = mybir.ActivationFunctionType

M = 128          # rows of x
K = 4096         # contraction
N = 11008        # out features
NCORES = 8
NLOC = N // NCORES           # 1376 columns per core
BLOCK_M, BLOCK_K = 32, 64
NBM, NBK = M // BLOCK_M, K // BLOCK_K   # 4 row blocks, 64 k blocks
KEEP = 32                               # k blocks kept per row block
NKT = K // 128                          # 32 k tiles of 128
N_TILES = [(0, 512), (512, 512), (1024, 352)]   # n-tile offsets/sizes

# W chunk schedule.  Queue order: gpsimd carries consts (75KB) then pure W;
# sync/scalar carry x (512KB each, in fine chunks so the mask pipeline
# starts early) then W.  k-tiles are assigned to chunks in expected arrival
# order so the GEMM (program order kt=0..31) is never blocked on a late
# chunk.  gpsimd's first chunk is small (it arrives first and unblocks the
# GEMM); queue byte totals are balanced so all three drain together.
# (queue, n_ktiles) in global arrival order; kt ranges assigned in order.
_SCHED = [("G", 2), ("S", 2), ("A", 2), ("G", 3), ("S", 2), ("A", 2),
          ("G", 3), ("S", 2), ("A", 2), ("G", 3), ("S", 2), ("A", 2),
          ("S", 2), ("A", 2), ("G", 1)]


def _program(ctx: ExitStack, tc: tile.TileContext, ins, outs):
    nc = tc.nc
    x_d, w_d, b_d, e_d, blob_d = ins
    (o_d,) = outs

    const = ctx.enter_context(tc.tile_pool(name="const", bufs=1))
    mk = ctx.enter_context(tc.tile_pool(name="mk", bufs=1))
    xc = ctx.enter_context(tc.tile_pool(name="xc", bufs=8))
    xtp = ctx.enter_context(tc.tile_pool(name="xtp", bufs=1))
    wpool = ctx.enter_context(tc.tile_pool(name="wpool", bufs=len(_SCHED)))
    xmpool = ctx.enter_context(tc.tile_pool(name="xmpool", bufs=8))
    opool = ctx.enter_context(tc.tile_pool(name="opool", bufs=1))
    psum = ctx.enter_context(tc.tile_pool(name="psum", bufs=1, space="PSUM"))

    # consts first on the gpsimd ring: blob = [ident | jh | ksel], then E,
    # bias.  ident leads because the PE transposes need it earliest.
    blob = const.tile([128, 288], F16)
    nc.gpsimd.dma_start(blob[:], blob_d)
    ident = blob[:, 0:128]
    jh = blob[0:64, 128:256]
    ksel = blob[0:64, 256:256 + NKT]
    e_sb = const.tile([128, NBM], F32)
    nc.gpsimd.dma_start(e_sb[:], e_d)
    bias_sb = const.tile([1, NLOC], F16)
    nc.gpsimd.dma_start(bias_sb[:], b_d)

    # ---- HAM warm-up: ~4us of junk matmuls so the PE clock-gate opens
    # before the transposes/GEMM start (otherwise everything runs at 1.2 GHz)
    warm_sb = mk.tile([128, 512], F16)
    nc.vector.memset(warm_sb[:], 0.0)
    warm_ps = psum.tile([128, 512], F32, name="warm_ps", tag="warm", bufs=1)
    for i in range(9):
        nc.tensor.matmul(warm_ps[:], lhsT=warm_sb[:, 0:128], rhs=warm_sb[:],
                         start=True, stop=True)

    # ---- x in 8 fine chunks alternating sync/scalar (4KB lines) so the
    # mask pipeline (reduce -> topk) starts as soon as the first chunk lands
    NCH = 8
    xw = K // NCH
    part_n = mk.tile([128, NBK], F32)
    jc = NBK // NCH
    x_cs = []
    for c in range(NCH):
        x_c = xc.tile([128, xw], F16, name=f"xch{c}", tag="xch")
        (nc.sync if c % 2 == 0 else nc.scalar).dma_start(
            x_c[:], x_d[:, c * xw:(c + 1) * xw])
        nc.vector.tensor_reduce(
            part_n[:, c * jc:(c + 1) * jc],
            x_c[:].rearrange("p (j k) -> p j k", k=BLOCK_K),
            axis=AX.X, op=ALU.add, apply_absolute_value=True)
        x_cs.append(x_c)

    # ---- W chunks (11KB+ per partition line) in expected arrival order
    eng = {"G": nc.gpsimd, "S": nc.sync, "A": nc.scalar}
    wch = []      # (tile, kt0, nkt)
    kt0 = 0
    for q, nkt in _SCHED:
        wt = wpool.tile([128, nkt * NLOC], F16, name=f"w{kt0}", tag="w")
        eng[q].dma_start(wt[:], w_d[:, kt0 * NLOC:(kt0 + nkt) * NLOC])
        wch.append((wt, kt0, nkt))
        kt0 += nkt
    kt_tile = [None] * NKT    # kt -> (tile, col offset)
    for wt, kt0, nkt in wch:
        for t in range(nkt):
            kt_tile[kt0 + t] = (wt, t * NLOC)

    # PE-transpose each 128-wide k tile; batch 4 per PSUM flush into xt_all
    xt_all = xtp.tile([128, K], F16)
    for tb in range(NKT // 4):
        tp = psum.tile([128, 512], F16, name=f"tp{tb}", tag="tp", bufs=2)
        for t in range(4):
            kt = 4 * tb + t
            src = x_cs[kt // 4]
            nc.tensor.transpose(tp[:, t * 128:(t + 1) * 128],
                                src[:, (kt % 4) * 128:(kt % 4 + 1) * 128],
                                ident)
        nc.vector.tensor_copy(xt_all[:, tb * 512:(tb + 1) * 512], tp[:])

    # ba_ps[b, j] = sum_m E[m, b] * part_n[m, j]  (block sums, b on partitions)
    ba_ps = psum.tile([NBM, NBK], F32, tag="mkps", bufs=2)
    nc.tensor.matmul(ba_ps[:], lhsT=e_sb[:], rhs=part_n[:], start=True, stop=True)

    # mean = sum / 2048 (exact power of two), rounded to f16 like jnp.mean
    ba16 = mk.tile([NBM, NBK], F16)
    nc.vector.tensor_scalar_mul(ba16[:], ba_ps[:], 1.0 / 2048.0)

    # arow[i, b*64+j] = a[b, j] on 64 partitions, via block-diag expand + matmul
    # rhs3[c, b*64+j] = a[c, j] * [c == b]
    rhs3 = mk.tile([NBM, NBM * NBK], F16)
    nc.vector.tensor_tensor(
        rhs3[:].rearrange("c (b j) -> c b j", b=NBM),
        ba16[:].unsqueeze(1).broadcast_to((NBM, NBM, NBK)),
        ident[0:NBM, 0:NBM].unsqueeze(-1).broadcast_to((NBM, NBM, NBK)),
        op=ALU.mult)
    ones4c = mk.tile([NBM, 64], F16)
    nc.vector.memset(ones4c[:], 1.0)
    arow_ps = psum.tile([64, NBM * NBK], F32, tag="mkps", bufs=2)
    nc.tensor.matmul(arow_ps[:], lhsT=ones4c[:], rhs=rhs3[:], start=True, stop=True)
    arow = mk.tile([64, NBM * NBK], F16)
    nc.vector.tensor_copy(arow[:], arow_ps[:])

    # acol[i, b] = a[b, i] via PE transpose
    acol_ps = psum.tile([64, NBM], F16, tag="mkps", bufs=2)
    nc.tensor.transpose(acol_ps[:], ba16[:], ident[0:NBM, 0:NBM])
    acol = mk.tile([64, NBM], F16)
    nc.vector.tensor_copy(acol[:], acol_ps[:])

    # cnt[i, b] = #{j : a[b, j] > a[b, i]};  keep iff cnt < KEEP
    cmp = mk.tile([64, NBM * NBK], F16)
    nc.vector.tensor_tensor(
        cmp[:].rearrange("i (b j) -> i b j", b=NBM),
        arow[:].rearrange("i (b j) -> i b j", b=NBM),
        acol[:].unsqueeze(-1).broadcast_to((64, NBM, NBK)),
        op=ALU.is_gt)
    cnt = mk.tile([64, NBM], F32)
    nc.vector.tensor_reduce(cnt[:], cmp[:].rearrange("i (b j) -> i b j", b=NBM),
                            axis=AX.X, op=ALU.add)
    keep16 = mk.tile([64, NBM], F16)
    nc.vector.tensor_scalar(keep16[:], cnt[:], float(KEEP), None, op0=ALU.is_lt)

    # keep2[p, kt*4+b] = keep16[2kt + p//64, b]  (kt-major so a 4-ktile xm
    # batch reads a contiguous 16-column slice)
    # rhs2[j, kt*4+b] = keep16[j, b] * Ksel[j, kt]
    rhs2 = mk.tile([64, 128], F16)
    nc.vector.tensor_tensor(
        rhs2[:].rearrange("j (kt b) -> j kt b", kt=NKT),
        keep16[:].unsqueeze(1).broadcast_to((64, NKT, NBM)),
        ksel[:].unsqueeze(-1).broadcast_to((64, NKT, NBM)),
        op=ALU.mult)
    ks_ps = psum.tile([128, 128], F32, tag="mkps", bufs=2)
    nc.tensor.matmul(ks_ps[:], lhsT=jh[:], rhs=rhs2[:], start=True, stop=True)
    keep2 = mk.tile([128, 128], F16)
    nc.vector.tensor_copy(keep2[:], ks_ps[:])

    ones = const.tile([1, 128], F16)
    nc.vector.memset(ones[:], 1.0)

    # ---- main GEMM: out[m, n] = sum_kt xm_kt.T @ w_kt + ones.T @ bias ----
    pbanks = [psum.tile([128, 512], F32, name=f"pn{i}", tag=f"pn{i}")
              for i in range(3)]
    # bias as the FIRST accumulation into each bank (start=True) so the
    # banks are complete right when the last k-tile matmul lands
    for nt, (n0, nsz) in enumerate(N_TILES):
        nc.tensor.matmul(pbanks[nt][:, :nsz], lhsT=ones[:],
                         rhs=bias_sb[:, n0:n0 + nsz], start=True, stop=False)

    # masked xT in 4-ktile batches: xm[p, q*32+m] = xt * keep2[p, q]
    xms = []
    for xb in range(NKT // 4):
        xm_b = xmpool.tile([128, 512], F16, name=f"xm{xb}", tag="xm")
        nc.vector.tensor_tensor(
            xm_b[:].rearrange("p (q m) -> p q m", m=BLOCK_M),
            xt_all[:, xb * 512:(xb + 1) * 512].rearrange(
                "p (q m) -> p q m", m=BLOCK_M),
            keep2[:, xb * 16:(xb + 1) * 16].unsqueeze(-1).broadcast_to(
                (128, 16, BLOCK_M)),
            op=ALU.mult)
        xms.append(xm_b)

    for kt in range(NKT):
        lhsT = xms[kt // 4][:, (kt % 4) * 128:(kt % 4 + 1) * 128]
        wt, wbase = kt_tile[kt]
        for nt, (n0, nsz) in enumerate(N_TILES):
            nc.tensor.matmul(pbanks[nt][:, :nsz],
                             lhsT=lhsT,
                             rhs=wt[:, wbase + n0:wbase + n0 + nsz],
                             start=False, stop=(kt == NKT - 1))

    out_sb = opool.tile([128, NLOC], F16)
    out_dma = [nc.sync, nc.scalar, nc.gpsimd]
    pi = 0
    for nt, (n0, nsz) in enumerate(N_TILES):
        for half in range(2):
            h0 = n0 + half * (nsz // 2)
            hsz = nsz // 2 if half == 0 else nsz - nsz // 2
            src = pbanks[nt][:, h0 - n0:h0 - n0 + hsz]
            dst = out_sb[:, h0:h0 + hsz]
            if pi % 2 == 0:
                nc.scalar.activation(dst, src, ACT.Copy)
            else:
                nc.vector.tensor_copy(dst, src)
            out_dma[pi % 3].dma_start(o_d[:, h0:h0 + hsz], dst)
            pi += 1


_CACHE = {}


def _build():
    if "nc" in _CACHE:
        return _CACHE["nc"]
    nc = bacc.Bacc("TRN2", target_bir_lowering=False, debug=False,
                   num_devices=NCORES)
    x_d = nc.dram_tensor("x", (M, K), F16, kind="ExternalInput").ap()
    w_d = nc.dram_tensor("w", (128, NKT * NLOC), F16, kind="ExternalInput").ap()
    b_d = nc.dram_tensor("bias", (1, NLOC), F16, kind="ExternalInput").ap()
    e_d = nc.dram_tensor("E", (M, NBM), F32, kind="ExternalInput").ap()
    blob_d = nc.dram_tensor("blob", (128, 288), F16, kind="ExternalInput").ap()
    o_d = nc.dram_tensor("out", (M, NLOC), F16, kind="ExternalOutput").ap()
    with tile.TileContext(nc) as tc:
        with ExitStack() as ctx:
            _program(ctx, tc, [x_d, w_d, b_d, e_d, blob_d], [o_d])
    nc.compile()
    _CACHE["nc"] = nc
    return nc


def _make_in_maps(x2, weight, bias):
    e_np = np.zeros((M, NBM), np.float32)
    for b in range(NBM):
        e_np[b * BLOCK_M:(b + 1) * BLOCK_M, b] = 1.0
    j_idx = np.arange(64)
    blob_np = np.zeros((128, 288), np.float16)
    blob_np[:, 0:128] = np.eye(128, dtype=np.float16)
    blob_np[0:64, 128:256] = (
        j_idx[:, None] % 2 == (np.arange(128)[None, :] // 64)).astype(np.float16)
    blob_np[0:64, 256:256 + NKT] = (
        j_idx[:, None] // 2 == np.arange(NKT)[None, :]).astype(np.float16)

    w16 = np.asarray(weight).astype(np.float16, copy=False)
    b16 = np.asarray(bias).astype(np.float16, copy=False)
    in_maps = []
    for c in range(NCORES):
        sl = slice(c * NLOC, (c + 1) * NLOC)
        # [K, NLOC] -> [128, NKT*NLOC]: row p holds W[kt*128+p, :] for all kt
        wp = np.ascontiguousarray(
            w16[:, sl].reshape(NKT, 128, NLOC).transpose(1, 0, 2).reshape(
                128, NKT * NLOC))
        in_maps.append({
            "x": x2,
            "w": wp,
            "bias": np.ascontiguousarray(b16[sl].reshape(1, NLOC)),
            "E": e_np,
            "blob": blob_np,
        })
    return in_maps


def kernel(x: np.ndarray, weight: np.ndarray, bias: np.ndarray) -> np.ndarray:
    x = np.asarray(x)
    weight = np.asarray(weight)
    bias = np.asarray(bias)
    bsz, seq, hidden = x.shape
    assert (bsz, seq, hidden) == (M, 1, K) and weight.shape == (K, N)

    x2 = np.ascontiguousarray(x.reshape(M, K).astype(np.float16, copy=False))
    in_maps = _make_in_maps(x2, weight, bias)
    nc = _build()
    res = run_bass_kernel_spmd(nc, in_maps, core_ids=list(range(NCORES)))
    out = np.concatenate([r["out"] for r in res.results], axis=1)
    return out.reshape(M, 1, N).astype(x.dtype, copy=False)


if __name__ == "__main__":
    rng = np.random.default_rng(0)
    x = rng.standard_normal((M, 1, K)).astype(np.float16)
    w = (rng.standard_normal((K, N)) * 0.01).astype(np.float16)
    b = np.zeros((N,), np.float16)
    out = kernel(x, w, b)
    print(out.shape, out.dtype)
